# revision 30
# baseline (speedup 1.0000x reference)
"""Fused MergedQKVParallelLinearWithDelta kernel for 8 Trainium2 NeuronCores.

Strategy (tensor-parallel on the QKV output dim, vLLM-style):
  - Each core owns a 768-row output shard (512 q + 128 k + 128 v rows).
  - Host pre-lays-out all weights K-major (no device-side weight
    transposes): w_base transposed to [IN, OS] f16; the GPTQ nibbles are
    repacked so that a single shift-and-mask extraction of plane jj from
    u16-word chunk C yields weights for the natural k-block 512C+128jj..+128.
  - Tokens are sorted by adapter on the host (no inter-adapter padding;
    T==1024 is already a multiple of 128).  The device gathers token rows
    with an indirect DMA (f32->f16 cast in flight) and transposes each
    128-token tile to K-major with ONE batched DMA-transpose instruction
    whose 3D output AP writes all 32 k-blocks at once.
  - Per adapter d the device builds merged weights
        wfull[k, o] = w_base[o, k] + sc[d, o] * w4[d, o, k]      (f16)
    (extract u16 on DVE, scale-multiply with partition-replicated scales
    that also converts to f16, base-add on GpSimd), so each token needs
    only ONE matmul pass over K.  Bias and the GPTQ zero-point correction
    -(z+1)*sc * colsum(x) enter as a single K=3 aux matmul per PSUM piece
    (rows: ones / u_hi / u_lo residual for f16 precision).
  - wfull is split into o-halves so building adapter d+1's half overlaps
    the matmuls of adapter d's other half (single-buffered tiles, WAR
    deps via the tile framework).
  - The host de-permutes/reassembles the 8 transposed output shards.
"""

import numpy as np

import concourse.bass as bass
import concourse.tile as tile
from concourse import bacc
from concourse import mybir
from concourse.bass_utils import run_bass_kernel_spmd

N_CORES = 8
T, IN = 1024, 4096
Q, KV = 4096, 1024
OUT = Q + 2 * KV
D = 4
OS = OUT // N_CORES          # 768 output rows per core
OSH = OS // 2                # 384, o-half
NB = IN // 128               # 32 k-blocks
NC_CHUNK = IN // 512         # 8 u16-word chunks of 128 words
N_TILES = T // 128           # 8 token tiles
T_PAD = T                    # no padding needed (T % 128 == 0)
UCHUNK = 512

F16 = mybir.dt.float16
F32 = mybir.dt.float32
U16 = mybir.dt.uint16
I32 = mybir.dt.int32


# ---------------------------------------------------------------------------
# Host-side routing schedule
# ---------------------------------------------------------------------------
def _schedule(indices):
    idx = np.asarray(indices).astype(np.int64)
    assert idx.shape == (T,)
    order = np.argsort(idx, kind="stable").astype(np.int32)
    counts = np.bincount(idx, minlength=D).astype(np.int64)
    return tuple(int(c) for c in counts), order


def _pieces(counts):
    """[(d, c0, c1)] column pieces (<=512 wide) in sorted-token space."""
    pieces = []
    c = 0
    for d in range(D):
        n = counts[d]
        if n == 0:
            continue
        p0 = c
        while p0 < c + n:
            p1 = min(p0 + 512, c + n)
            pieces.append((d, p0, p1))
            p0 = p1
        c += n
    return pieces


# ---------------------------------------------------------------------------
# Device program
# ---------------------------------------------------------------------------
def _build_program(counts):
    pieces = _pieces(counts)
    adapters = [d for d in range(D) if counts[d] > 0]

    nc = bacc.Bacc(
        trn_type="TRN2", target_bir_lowering=False, debug=False, num_devices=1
    )
    x_d = nc.dram_tensor("x", [T, IN], F16, kind="ExternalInput").ap()
    gidx_d = nc.dram_tensor("gidx", [128, N_TILES], I32, kind="ExternalInput").ap()
    wbT_d = nc.dram_tensor("wbT", [IN, OS], F16, kind="ExternalInput").ap()
    qwT_d = nc.dram_tensor(
        "qwT", [D, 2, IN // 4, OSH], U16, kind="ExternalInput"
    ).ap()
    scq_d = nc.dram_tensor(
        "scq", [D, 2, 128, 4 * OSH], F16, kind="ExternalInput"
    ).ap()
    auxw_d = nc.dram_tensor("auxw", [3, D * OS], F16, kind="ExternalInput").ap()
    outT_d = nc.dram_tensor("outT", [OS, T_PAD], F32, kind="ExternalOutput").ap()

    with tile.TileContext(nc) as tc:
        with (
            tc.tile_pool(name="misc", bufs=1) as pmisc,
            tc.tile_pool(name="big", bufs=1) as pbig,
            tc.tile_pool(name="gxp", bufs=2) as pgx,
            tc.tile_pool(name="qtp", bufs=2) as pqt,
            tc.tile_pool(name="scp", bufs=2) as psc,
            tc.tile_pool(name="axp", bufs=2) as pax,
            tc.tile_pool(name="oop", bufs=2) as poo,
            tc.tile_pool(name="urp", bufs=1) as pur,
            tc.tile_pool(name="pp", bufs=6, space="PSUM") as pps,
            tc.tile_pool(name="pu", bufs=2, space="PSUM") as ppu,
        ):
            # ---- constants / persistent tiles
            gidx = pmisc.tile([128, N_TILES], I32, tag="gidx")
            nc.gpsimd.dma_start(gidx[:], gidx_d[:])
            wb = pbig.tile([128, NB * OS], F16, tag="wb")
            ones_col = pmisc.tile([128, 1], F16, tag="onesc")
            nc.vector.memset(ones_col[:], 1.0)
            aux_x = pmisc.tile([3, T_PAD], F16, tag="auxx")
            nc.vector.memset(aux_x[0:1, :], 1.0)

            xgT = pbig.tile([128, NB * T_PAD], F16, tag="xgT")
            xgT3 = xgT[:].rearrange("p (kb t) -> p kb t", kb=NB)

            def emit_phase_a():
                # gather + batched transpose per token tile
                for ti in range(N_TILES):
                    gx = pgx.tile([128, IN], F16, tag="gx")
                    # split each tile gather into 4 column-slice gathers so
                    # the descriptors spread across DMA rings (one big
                    # indirect gather serializes on a single engine)
                    for gs in range(4):
                        nc.gpsimd.indirect_dma_start(
                            out=gx[:, gs * 1024 : (gs + 1) * 1024],
                            out_offset=None,
                            in_=x_d[:],
                            in_offset=bass.IndirectOffsetOnAxis(
                                ap=gidx[:, ti : ti + 1], axis=0
                            ),
                            element_offset=gs * 1024,
                        )
                    # NB: all transposes must go on ONE queue — concurrent
                    # DMA-transposes from two queues clobber shared XBAR state.
                    nc.sync.dma_start(
                        xgT3[:, :, ti * 128 : (ti + 1) * 128],
                        gx[:],
                        transpose=True,
                    )

            # ---- per-adapter merged-weight build (o-halves), then matmuls.
            # wfh[h][p, kb*OSH + o] = w_base + sc*w4 for out col o of half h.
            # Build: (1) nibble-extract straight into the f16 tile as exact
            # subnormal bit patterns n*2^-18 (u32 lanes process 2 words/op),
            # (2) in-place all-f16 multiply by sc*2^18 per C-quad (2x DVE),
            # (3) in-place all-f16 add of w_base (DVE + 2 GpSimd per half).
            wfh = [
                pbig.tile([128, NB * OSH], F16, tag=f"wfh{h}", name=f"wfh{h}")
                for h in range(2)
            ]
            U32 = mybir.dt.uint32
            SHIFTS = {
                0: (mybir.AluOpType.logical_shift_left, 6),
                1: (mybir.AluOpType.logical_shift_left, 2),
                2: (mybir.AluOpType.logical_shift_right, 2),
                3: (mybir.AluOpType.logical_shift_right, 6),
            }
            wb3 = wb[:].rearrange("p (kb o) -> p kb o", kb=NB)

            u_psums = {}

            def emit_u_psum(ci):
                c0 = ci * UCHUNK
                c1 = min(c0 + UCHUNK, T_PAD)
                clen = c1 - c0
                up = ppu.tile([1, UCHUNK], F32, space="PSUM", tag="up")
                for kb in range(NB):
                    nc.tensor.matmul(
                        up[:, 0:clen],
                        lhsT=ones_col[:],
                        rhs=xgT[:, kb * T_PAD + c0 : kb * T_PAD + c1],
                        start=(kb == 0),
                        stop=(kb == NB - 1),
                    )
                u_psums[ci] = up

            def emit_u_copy(ci):
                c0 = ci * UCHUNK
                c1 = min(c0 + UCHUNK, T_PAD)
                clen = c1 - c0
                up = u_psums[ci]
                ur = pur.tile([1, UCHUNK], F16, tag="ur")
                url = pur.tile([1, UCHUNK], F16, tag="url")
                nc.vector.tensor_copy(ur[:, 0:clen], up[:, 0:clen])
                nc.vector.tensor_tensor(
                    out=url[:, 0:clen],
                    in0=up[:, 0:clen],
                    in1=ur[:, 0:clen],
                    op=mybir.AluOpType.subtract,
                )
                nc.scalar.dma_start(aux_x[1:2, c0:c1], ur[:, 0:clen])
                nc.scalar.dma_start(aux_x[2:3, c0:c1], url[:, 0:clen])

            def emit_loads(d, h):
                qth = pqt.tile([128, NC_CHUNK * OSH], U16, tag="qth")
                nc.scalar.dma_start(
                    qth[:].rearrange("p (C o) -> p C o", C=NC_CHUNK),
                    qwT_d[d, h].rearrange("(C p) o -> p C o", p=128),
                )
                scq = psc.tile([128, 4 * OSH], F16, tag="scq")
                nc.scalar.dma_start(scq[:], scq_d[d, h])
                return qth, scq

            def emit_build(d, h, loads=None):
                qth, scq = loads if loads is not None else emit_loads(d, h)
                for C in range(NC_CHUNK):
                    for jj in range(4):
                        kb = 4 * C + jj
                        op0, sh = SHIFTS[jj]
                        nc.vector.tensor_scalar(
                            out=wfh[h][:, kb * OSH : (kb + 1) * OSH].bitcast(
                                U32
                            ),
                            in0=qth[:, C * OSH : (C + 1) * OSH].bitcast(U32),
                            scalar1=sh,
                            scalar2=0x03C003C0,
                            op0=op0,
                            op1=mybir.AluOpType.bitwise_and,
                        )
                for C in range(NC_CHUNK):
                    quad = wfh[h][:, 4 * C * OSH : (4 * C + 4) * OSH]
                    nc.vector.tensor_tensor(
                        out=quad, in0=quad, in1=scq[:],
                        op=mybir.AluOpType.mult,
                    )
                    nc.vector.tensor_tensor(
                        out=quad,
                        in0=quad,
                        in1=wb3[:, 4 * C : 4 * C + 4, h * OSH : (h + 1) * OSH],
                        op=mybir.AluOpType.add,
                    )

            def emit_te_mains(d, h):
                dp = [p for p in pieces if p[0] == d]
                groups = []
                for lot in range(3):
                    pss = []
                    for _ in dp:
                        ps = pps.tile([128, 512], F32, space="PSUM", tag="pp")
                        pss.append(ps)
                    for kb in range(NB):
                        for pi, (_, pc0, pc1) in enumerate(dp):
                            nc.tensor.matmul(
                                pss[pi][:, 0 : pc1 - pc0],
                                lhsT=wfh[h][
                                    :,
                                    kb * OSH + lot * 128 : kb * OSH
                                    + (lot + 1) * 128,
                                ],
                                rhs=xgT[:, kb * T_PAD + pc0 : kb * T_PAD + pc1],
                                start=(kb == 0),
                                stop=False,
                            )
                    groups.append(pss)
                return groups

            def emit_te_aux(d, h, auxw, groups):
                dp = [p for p in pieces if p[0] == d]
                for lot in range(3):
                    ot = 3 * h + lot
                    pss = groups[lot]
                    for pi, (_, pc0, pc1) in enumerate(dp):
                        plen = pc1 - pc0
                        nc.tensor.matmul(
                            pss[pi][:, 0:plen],
                            lhsT=auxw[0:3, ot * 128 : (ot + 1) * 128],
                            rhs=aux_x[0:3, pc0:pc1],
                            start=False,
                            stop=True,
                        )
                        oo = poo.tile([128, 512], F32, tag="oo")
                        nc.scalar.copy(oo[:, 0:plen], pss[pi][:, 0:plen])
                        nc.scalar.dma_start(
                            outT_d[ot * 128 : (ot + 1) * 128, pc0:pc1],
                            oo[:, 0:plen],
                        )

            def load_auxw(d):
                auxw = pax.tile([3, OS], F16, tag="auxw")
                nc.scalar.dma_start(auxw[:], auxw_d[:, d * OS : (d + 1) * OS])
                return auxw

            # Schedule. d0's qth/scq loads go on the gpsimd queue BEFORE the
            # token gathers (in-order queue — otherwise they sit behind
            # gathers that are themselves WAR-blocked on late transposes).
            # u colsum chunks are emitted lazily, just before the first
            # adapter whose aux matmuls need them — chunk 1 depends on the
            # LAST transposes, and emitting it early head-blocks both the
            # vector queue (ur/url copies) and the tensor queue.
            dcol = {}
            c = 0
            for d in range(D):
                if counts[d] > 0:
                    dcol[d] = (c, c + counts[d])
                    c += counts[d]
            n_chunks = (T_PAD + UCHUNK - 1) // UCHUNK
            done_chunks = set()

            d0 = adapters[0]
            l00 = emit_loads(d0, 0)
            l01 = emit_loads(d0, 1)
            auxw0 = load_auxw(d0)
            # wb[p, kb*OS + o] = wbT[kb*128 + p, o] — on the scalar queue
            # after the small d0 preloads (it is only needed by the first
            # build's add pass, ~50us in)
            nc.scalar.dma_start(
                wb[:].rearrange("p (kb o) -> p kb o", kb=NB),
                wbT_d[:].rearrange("(kb p) o -> p kb o", p=128),
            )
            emit_phase_a()
            first = True
            for d in adapters:
                if first:
                    auxw = auxw0
                    emit_build(d, 0, l00)
                    g0 = emit_te_mains(d, 0)
                    emit_build(d, 1, l01)
                    g1 = emit_te_mains(d, 1)
                    first = False
                else:
                    auxw = load_auxw(d)
                    emit_build(d, 0)
                    g0 = emit_te_mains(d, 0)
                    emit_build(d, 1)
                    g1 = emit_te_mains(d, 1)
                for ci in range(n_chunks):
                    if ci in done_chunks:
                        continue
                    c0, c1 = dcol[d]
                    if c0 < (ci + 1) * UCHUNK and c1 > ci * UCHUNK:
                        emit_u_psum(ci)
                        emit_u_copy(ci)
                        done_chunks.add(ci)
                emit_te_aux(d, 0, auxw, g0)
                emit_te_aux(d, 1, auxw, g1)
    nc.compile()
    return nc


# ---------------------------------------------------------------------------
# Host-side data prep
# ---------------------------------------------------------------------------
def _unpack_zeros(qz, o_count):
    o = np.arange(o_count)
    words = qz[:, o >> 3, 0].astype(np.int64)
    return ((words >> (4 * (o & 7))) & 0xF).astype(np.float32)


def _repack_qw(qw_c):
    """[D, OS, IN//8] int32 -> [D, 2, IN//4, OSH] u16 such that extracting
    nibble-plane jj (shift 4*jj) from word row 128*C+p of half h gives the
    f16 weight for k = 512*C + 128*jj + p, output col o (within half h)."""
    D_, O_, KW = qw_c.shape
    w = qw_c.view(np.uint32)
    shifts8 = (4 * np.arange(8, dtype=np.uint32)).reshape(1, 1, 1, 8)
    nib = ((w[:, :, :, None] >> shifts8) & 0xF).astype(np.uint16)  # [D,O,KW,8]
    nib = nib.reshape(D_, O_, KW * 8)  # k = kw*8 + j
    v = nib.reshape(D_, O_, NC_CHUNK, 4, 128)  # [d, o, C, jj, p]
    shifts4 = (4 * np.arange(4, dtype=np.uint16)).reshape(1, 1, 1, 4, 1)
    words = (
        (v.astype(np.uint32) << shifts4.astype(np.uint32)).sum(axis=3) & 0xFFFF
    ).astype(np.uint16)  # [d, o, C, p]
    words = words.transpose(0, 2, 3, 1).reshape(D_, IN // 4, O_)  # [(C,p), o]
    halves = np.stack([words[:, :, :OSH], words[:, :, OSH:]], axis=1)
    return np.ascontiguousarray(halves)


_prog_cache = {}


def kernel(**inputs):
    x = np.ascontiguousarray(np.asarray(inputs["x"], dtype=np.float32).astype(np.float16))
    w_base = np.asarray(inputs["w_base"], dtype=np.float32)
    bias = np.asarray(inputs["bias"], dtype=np.float32)
    qw_q = np.asarray(inputs["qweight_q"], dtype=np.int32)
    qw_k = np.asarray(inputs["qweight_k"], dtype=np.int32)
    qw_v = np.asarray(inputs["qweight_v"], dtype=np.int32)
    qz_q = np.asarray(inputs["qzeros_q"], dtype=np.int32)
    qz_k = np.asarray(inputs["qzeros_k"], dtype=np.int32)
    qz_v = np.asarray(inputs["qzeros_v"], dtype=np.int32)
    sc_q = np.asarray(inputs["scales_q"], dtype=np.float32)
    sc_k = np.asarray(inputs["scales_k"], dtype=np.float32)
    sc_v = np.asarray(inputs["scales_v"], dtype=np.float32)
    indices = np.asarray(inputs["indices"])

    counts, order = _schedule(indices)

    if counts not in _prog_cache:
        _prog_cache[counts] = _build_program(counts)
    nc = _prog_cache[counts]

    z_q = _unpack_zeros(qz_q, Q)
    z_k = _unpack_zeros(qz_k, KV)
    z_v = _unpack_zeros(qz_v, KV)

    gidx_host = np.ascontiguousarray(order.reshape(N_TILES, 128).T)

    SQ, SK = Q // N_CORES, KV // N_CORES
    in_maps = []
    for c in range(N_CORES):
        qs = slice(SQ * c, SQ * (c + 1))
        ks = slice(SK * c, SK * (c + 1))
        wb = np.concatenate(
            [w_base[qs], w_base[Q + SK * c : Q + SK * (c + 1)],
             w_base[Q + KV + SK * c : Q + KV + SK * (c + 1)]], axis=0
        )  # [OS, IN]
        qw = np.concatenate([qw_q[:, qs], qw_k[:, ks], qw_v[:, ks]], axis=1)
        z = np.concatenate([z_q[:, qs], z_k[:, ks], z_v[:, ks]], axis=1)
        sc = np.concatenate(
            [sc_q[:, qs, 0], sc_k[:, ks, 0], sc_v[:, ks, 0]], axis=1
        )  # [D, OS]
        b = np.concatenate(
            [bias[qs], bias[Q + SK * c : Q + SK * (c + 1)],
             bias[Q + KV + SK * c : Q + KV + SK * (c + 1)]]
        )  # [OS]

        wbT = np.ascontiguousarray(wb.T.astype(np.float16))  # [IN, OS]
        qwT = _repack_qw(np.ascontiguousarray(qw))
        # scq[d, h, p, q*OSH + o] = sc[d, h*OSH + o] * 2^18 (per C-quad mult)
        s18 = (sc * float(2.0**18)).astype(np.float16)  # [D, OS]
        scq = np.empty([D, 2, 128, 4 * OSH], np.float16)
        for d in range(D):
            for h in range(2):
                row = np.tile(s18[d, h * OSH : (h + 1) * OSH], 4)
                scq[d, h] = row[None, :]
        scq = np.ascontiguousarray(scq)
        znr2 = (-(z + 1.0) * sc).astype(np.float16)  # [D, OS]
        auxw = np.zeros([3, D * OS], np.float16)
        auxw[0] = np.tile(b.astype(np.float16), D)
        auxw[1] = znr2.reshape(-1)
        auxw[2] = znr2.reshape(-1)

        in_maps.append(
            {
                "x": x,
                "gidx": gidx_host,
                "wbT": wbT,
                "qwT": qwT,
                "scq": scq,
                "auxw": np.ascontiguousarray(auxw),
            }
        )

    import os

    trace = bool(int(os.environ.get("KERNEL_TRACE", "0")))
    res = run_bass_kernel_spmd(
        nc, in_maps, core_ids=list(range(N_CORES)), trace=trace
    )
    kernel._last_results = res

    out = np.zeros([T, OUT], np.float32)
    for c in range(N_CORES):
        rT = np.asarray(res.results[c]["outT"])  # [OS, T_PAD]
        r = rT.T  # [T_PAD, OS]
        cols = np.concatenate(
            [
                np.arange(SQ * c, SQ * (c + 1)),
                np.arange(Q + SK * c, Q + SK * (c + 1)),
                np.arange(Q + KV + SK * c, Q + KV + SK * (c + 1)),
            ]
        )
        out[order[:, None], cols[None, :]] = r
    return out


# revision 31
# speedup vs baseline: 1.0165x; 1.0165x over previous
"""Fused MergedQKVParallelLinearWithDelta kernel for 8 Trainium2 NeuronCores.

Strategy (tensor-parallel on the QKV output dim, vLLM-style):
  - Each core owns a 768-row output shard (512 q + 128 k + 128 v rows).
  - Host pre-lays-out all weights K-major (no device-side weight
    transposes): w_base transposed to [IN, OS] f16; the GPTQ nibbles are
    repacked so that a single shift-and-mask extraction of plane jj from
    u16-word chunk C yields weights for the natural k-block 512C+128jj..+128.
  - Tokens are sorted by adapter on the host (no inter-adapter padding;
    T==1024 is already a multiple of 128).  The device gathers token rows
    with an indirect DMA (f32->f16 cast in flight) and transposes each
    128-token tile to K-major with ONE batched DMA-transpose instruction
    whose 3D output AP writes all 32 k-blocks at once.
  - Per adapter d the device builds merged weights
        wfull[k, o] = w_base[o, k] + sc[d, o] * w4[d, o, k]      (f16)
    (extract u16 on DVE, scale-multiply with partition-replicated scales
    that also converts to f16, base-add on GpSimd), so each token needs
    only ONE matmul pass over K.  Bias and the GPTQ zero-point correction
    -(z+1)*sc * colsum(x) enter as a single K=3 aux matmul per PSUM piece
    (rows: ones / u_hi / u_lo residual for f16 precision).
  - wfull is split into o-halves so building adapter d+1's half overlaps
    the matmuls of adapter d's other half (single-buffered tiles, WAR
    deps via the tile framework).
  - The host de-permutes/reassembles the 8 transposed output shards.
"""

import numpy as np

import concourse.bass as bass
import concourse.tile as tile
from concourse import bacc
from concourse import mybir
from concourse.bass_utils import run_bass_kernel_spmd

N_CORES = 8
T, IN = 1024, 4096
Q, KV = 4096, 1024
OUT = Q + 2 * KV
D = 4
OS = OUT // N_CORES          # 768 output rows per core
OSH = OS // 2                # 384, o-half
NB = IN // 128               # 32 k-blocks
NC_CHUNK = IN // 512         # 8 u16-word chunks of 128 words
N_TILES = T // 128           # 8 token tiles
T_PAD = T                    # no padding needed (T % 128 == 0)
UCHUNK = 512

F16 = mybir.dt.float16
F32 = mybir.dt.float32
U16 = mybir.dt.uint16
I32 = mybir.dt.int32


# ---------------------------------------------------------------------------
# Host-side routing schedule
# ---------------------------------------------------------------------------
def _schedule(indices):
    idx = np.asarray(indices).astype(np.int64)
    assert idx.shape == (T,)
    order = np.argsort(idx, kind="stable").astype(np.int32)
    counts = np.bincount(idx, minlength=D).astype(np.int64)
    return tuple(int(c) for c in counts), order


def _pieces(counts):
    """[(d, c0, c1)] column pieces (<=512 wide) in sorted-token space."""
    pieces = []
    c = 0
    for d in range(D):
        n = counts[d]
        if n == 0:
            continue
        p0 = c
        while p0 < c + n:
            p1 = min(p0 + 512, c + n)
            pieces.append((d, p0, p1))
            p0 = p1
        c += n
    return pieces


# ---------------------------------------------------------------------------
# Device program
# ---------------------------------------------------------------------------
def _build_program(counts):
    pieces = _pieces(counts)
    adapters = [d for d in range(D) if counts[d] > 0]

    nc = bacc.Bacc(
        trn_type="TRN2", target_bir_lowering=False, debug=False, num_devices=1
    )
    # x arrives host-sorted by adapter (and cast to f16): plain DRAM reads
    x_d = nc.dram_tensor("x", [T, IN], F16, kind="ExternalInput").ap()
    wbT_d = nc.dram_tensor("wbT", [IN, OS], F16, kind="ExternalInput").ap()
    qwT_d = nc.dram_tensor(
        "qwT", [D, 2, IN // 4, OSH], U16, kind="ExternalInput"
    ).ap()
    scq_d = nc.dram_tensor(
        "scq", [D, 2, 128, 4 * OSH], F16, kind="ExternalInput"
    ).ap()
    auxw_d = nc.dram_tensor("auxw", [3, D * OS], F16, kind="ExternalInput").ap()
    outT_d = nc.dram_tensor("outT", [OS, T_PAD], F32, kind="ExternalOutput").ap()

    with tile.TileContext(nc) as tc:
        with (
            tc.tile_pool(name="misc", bufs=1) as pmisc,
            tc.tile_pool(name="big", bufs=1) as pbig,
            tc.tile_pool(name="qtp", bufs=2) as pqt,
            tc.tile_pool(name="scp", bufs=2) as psc,
            tc.tile_pool(name="axp", bufs=2) as pax,
            tc.tile_pool(name="oop", bufs=2) as poo,
            tc.tile_pool(name="urp", bufs=1) as pur,
            tc.tile_pool(name="pp", bufs=6, space="PSUM") as pps,
            tc.tile_pool(name="pu", bufs=2, space="PSUM") as ppu,
        ):
            # ---- constants / persistent tiles
            wb = pbig.tile([128, NB * OS], F16, tag="wb")
            ones_col = pmisc.tile([128, 1], F16, tag="onesc")
            nc.vector.memset(ones_col[:], 1.0)
            aux_x = pmisc.tile([3, T_PAD], F16, tag="auxx")
            nc.vector.memset(aux_x[0:1, :], 1.0)

            xgT = pbig.tile([128, NB * T_PAD], F16, tag="xgT")
            xgT3 = xgT[:].rearrange("p (kb t) -> p kb t", kb=NB)

            def emit_phase_a():
                # batched DMA-transpose straight from (host-sorted, f16) DRAM
                # x into K-major SBUF tiles; one instruction per token tile.
                # NB: all transposes must go on ONE queue — concurrent
                # DMA-transposes from two queues clobber shared XBAR state.
                for ti in range(N_TILES):
                    nc.sync.dma_start(
                        xgT3[:, :, ti * 128 : (ti + 1) * 128],
                        x_d[ti * 128 : (ti + 1) * 128, :],
                        transpose=True,
                    )

            # ---- per-adapter merged-weight build (o-halves), then matmuls.
            # wfh[h][p, kb*OSH + o] = w_base + sc*w4 for out col o of half h.
            # Build: (1) nibble-extract straight into the f16 tile as exact
            # subnormal bit patterns n*2^-18 (u32 lanes process 2 words/op),
            # (2) in-place all-f16 multiply by sc*2^18 per C-quad (2x DVE),
            # (3) in-place all-f16 add of w_base (DVE + 2 GpSimd per half).
            wfh = [
                pbig.tile([128, NB * OSH], F16, tag=f"wfh{h}", name=f"wfh{h}")
                for h in range(2)
            ]
            U32 = mybir.dt.uint32
            SHIFTS = {
                0: (mybir.AluOpType.logical_shift_left, 6),
                1: (mybir.AluOpType.logical_shift_left, 2),
                2: (mybir.AluOpType.logical_shift_right, 2),
                3: (mybir.AluOpType.logical_shift_right, 6),
            }
            wb3 = wb[:].rearrange("p (kb o) -> p kb o", kb=NB)

            u_psums = {}

            def emit_u_psum(ci):
                c0 = ci * UCHUNK
                c1 = min(c0 + UCHUNK, T_PAD)
                clen = c1 - c0
                up = ppu.tile([1, UCHUNK], F32, space="PSUM", tag="up")
                for kb in range(NB):
                    nc.tensor.matmul(
                        up[:, 0:clen],
                        lhsT=ones_col[:],
                        rhs=xgT[:, kb * T_PAD + c0 : kb * T_PAD + c1],
                        start=(kb == 0),
                        stop=(kb == NB - 1),
                    )
                u_psums[ci] = up

            def emit_u_copy(ci):
                c0 = ci * UCHUNK
                c1 = min(c0 + UCHUNK, T_PAD)
                clen = c1 - c0
                up = u_psums[ci]
                ur = pur.tile([1, UCHUNK], F16, tag="ur")
                url = pur.tile([1, UCHUNK], F16, tag="url")
                nc.vector.tensor_copy(ur[:, 0:clen], up[:, 0:clen])
                nc.vector.tensor_tensor(
                    out=url[:, 0:clen],
                    in0=up[:, 0:clen],
                    in1=ur[:, 0:clen],
                    op=mybir.AluOpType.subtract,
                )
                nc.scalar.dma_start(aux_x[1:2, c0:c1], ur[:, 0:clen])
                nc.scalar.dma_start(aux_x[2:3, c0:c1], url[:, 0:clen])

            def emit_loads(d, h):
                qth = pqt.tile([128, NC_CHUNK * OSH], U16, tag="qth")
                nc.scalar.dma_start(
                    qth[:].rearrange("p (C o) -> p C o", C=NC_CHUNK),
                    qwT_d[d, h].rearrange("(C p) o -> p C o", p=128),
                )
                scq = psc.tile([128, 4 * OSH], F16, tag="scq")
                nc.scalar.dma_start(scq[:], scq_d[d, h])
                return qth, scq

            def emit_build(d, h, loads=None):
                qth, scq = loads if loads is not None else emit_loads(d, h)
                for C in range(NC_CHUNK):
                    for jj in range(4):
                        kb = 4 * C + jj
                        op0, sh = SHIFTS[jj]
                        nc.vector.tensor_scalar(
                            out=wfh[h][:, kb * OSH : (kb + 1) * OSH].bitcast(
                                U32
                            ),
                            in0=qth[:, C * OSH : (C + 1) * OSH].bitcast(U32),
                            scalar1=sh,
                            scalar2=0x03C003C0,
                            op0=op0,
                            op1=mybir.AluOpType.bitwise_and,
                        )
                for C in range(NC_CHUNK):
                    quad = wfh[h][:, 4 * C * OSH : (4 * C + 4) * OSH]
                    nc.vector.tensor_tensor(
                        out=quad, in0=quad, in1=scq[:],
                        op=mybir.AluOpType.mult,
                    )
                    nc.vector.tensor_tensor(
                        out=quad,
                        in0=quad,
                        in1=wb3[:, 4 * C : 4 * C + 4, h * OSH : (h + 1) * OSH],
                        op=mybir.AluOpType.add,
                    )

            def emit_te_mains(d, h):
                dp = [p for p in pieces if p[0] == d]
                groups = []
                for lot in range(3):
                    pss = []
                    for _ in dp:
                        ps = pps.tile([128, 512], F32, space="PSUM", tag="pp")
                        pss.append(ps)
                    for kb in range(NB):
                        for pi, (_, pc0, pc1) in enumerate(dp):
                            nc.tensor.matmul(
                                pss[pi][:, 0 : pc1 - pc0],
                                lhsT=wfh[h][
                                    :,
                                    kb * OSH + lot * 128 : kb * OSH
                                    + (lot + 1) * 128,
                                ],
                                rhs=xgT[:, kb * T_PAD + pc0 : kb * T_PAD + pc1],
                                start=(kb == 0),
                                stop=False,
                            )
                    groups.append(pss)
                return groups

            def emit_te_aux(d, h, auxw, groups):
                dp = [p for p in pieces if p[0] == d]
                for lot in range(3):
                    ot = 3 * h + lot
                    pss = groups[lot]
                    for pi, (_, pc0, pc1) in enumerate(dp):
                        plen = pc1 - pc0
                        nc.tensor.matmul(
                            pss[pi][:, 0:plen],
                            lhsT=auxw[0:3, ot * 128 : (ot + 1) * 128],
                            rhs=aux_x[0:3, pc0:pc1],
                            start=False,
                            stop=True,
                        )
                        oo = poo.tile([128, 512], F32, tag="oo")
                        nc.scalar.copy(oo[:, 0:plen], pss[pi][:, 0:plen])
                        nc.scalar.dma_start(
                            outT_d[ot * 128 : (ot + 1) * 128, pc0:pc1],
                            oo[:, 0:plen],
                        )

            def load_auxw(d):
                auxw = pax.tile([3, OS], F16, tag="auxw")
                nc.scalar.dma_start(auxw[:], auxw_d[:, d * OS : (d + 1) * OS])
                return auxw

            # Schedule. d0's qth/scq loads go on the gpsimd queue BEFORE the
            # token gathers (in-order queue — otherwise they sit behind
            # gathers that are themselves WAR-blocked on late transposes).
            # u colsum chunks are emitted lazily, just before the first
            # adapter whose aux matmuls need them — chunk 1 depends on the
            # LAST transposes, and emitting it early head-blocks both the
            # vector queue (ur/url copies) and the tensor queue.
            dcol = {}
            c = 0
            for d in range(D):
                if counts[d] > 0:
                    dcol[d] = (c, c + counts[d])
                    c += counts[d]
            n_chunks = (T_PAD + UCHUNK - 1) // UCHUNK
            done_chunks = set()

            d0 = adapters[0]
            l00 = emit_loads(d0, 0)
            l01 = emit_loads(d0, 1)
            auxw0 = load_auxw(d0)
            # wb[p, kb*OS + o] = wbT[kb*128 + p, o] — on the scalar queue
            # after the small d0 preloads (it is only needed by the first
            # build's add pass, ~50us in)
            nc.scalar.dma_start(
                wb[:].rearrange("p (kb o) -> p kb o", kb=NB),
                wbT_d[:].rearrange("(kb p) o -> p kb o", p=128),
            )
            emit_phase_a()
            first = True
            for d in adapters:
                if first:
                    auxw = auxw0
                    emit_build(d, 0, l00)
                    g0 = emit_te_mains(d, 0)
                    emit_build(d, 1, l01)
                    g1 = emit_te_mains(d, 1)
                    first = False
                else:
                    auxw = load_auxw(d)
                    emit_build(d, 0)
                    g0 = emit_te_mains(d, 0)
                    emit_build(d, 1)
                    g1 = emit_te_mains(d, 1)
                for ci in range(n_chunks):
                    if ci in done_chunks:
                        continue
                    c0, c1 = dcol[d]
                    if c0 < (ci + 1) * UCHUNK and c1 > ci * UCHUNK:
                        emit_u_psum(ci)
                        emit_u_copy(ci)
                        done_chunks.add(ci)
                emit_te_aux(d, 0, auxw, g0)
                emit_te_aux(d, 1, auxw, g1)
    nc.compile()
    return nc


# ---------------------------------------------------------------------------
# Host-side data prep
# ---------------------------------------------------------------------------
def _unpack_zeros(qz, o_count):
    o = np.arange(o_count)
    words = qz[:, o >> 3, 0].astype(np.int64)
    return ((words >> (4 * (o & 7))) & 0xF).astype(np.float32)


def _repack_qw(qw_c):
    """[D, OS, IN//8] int32 -> [D, 2, IN//4, OSH] u16 such that extracting
    nibble-plane jj (shift 4*jj) from word row 128*C+p of half h gives the
    f16 weight for k = 512*C + 128*jj + p, output col o (within half h)."""
    D_, O_, KW = qw_c.shape
    w = qw_c.view(np.uint32)
    shifts8 = (4 * np.arange(8, dtype=np.uint32)).reshape(1, 1, 1, 8)
    nib = ((w[:, :, :, None] >> shifts8) & 0xF).astype(np.uint16)  # [D,O,KW,8]
    nib = nib.reshape(D_, O_, KW * 8)  # k = kw*8 + j
    v = nib.reshape(D_, O_, NC_CHUNK, 4, 128)  # [d, o, C, jj, p]
    shifts4 = (4 * np.arange(4, dtype=np.uint16)).reshape(1, 1, 1, 4, 1)
    words = (
        (v.astype(np.uint32) << shifts4.astype(np.uint32)).sum(axis=3) & 0xFFFF
    ).astype(np.uint16)  # [d, o, C, p]
    words = words.transpose(0, 2, 3, 1).reshape(D_, IN // 4, O_)  # [(C,p), o]
    halves = np.stack([words[:, :, :OSH], words[:, :, OSH:]], axis=1)
    return np.ascontiguousarray(halves)


_prog_cache = {}


def kernel(**inputs):
    x = np.asarray(inputs["x"], dtype=np.float32)
    w_base = np.asarray(inputs["w_base"], dtype=np.float32)
    bias = np.asarray(inputs["bias"], dtype=np.float32)
    qw_q = np.asarray(inputs["qweight_q"], dtype=np.int32)
    qw_k = np.asarray(inputs["qweight_k"], dtype=np.int32)
    qw_v = np.asarray(inputs["qweight_v"], dtype=np.int32)
    qz_q = np.asarray(inputs["qzeros_q"], dtype=np.int32)
    qz_k = np.asarray(inputs["qzeros_k"], dtype=np.int32)
    qz_v = np.asarray(inputs["qzeros_v"], dtype=np.int32)
    sc_q = np.asarray(inputs["scales_q"], dtype=np.float32)
    sc_k = np.asarray(inputs["scales_k"], dtype=np.float32)
    sc_v = np.asarray(inputs["scales_v"], dtype=np.float32)
    indices = np.asarray(inputs["indices"])

    counts, order = _schedule(indices)
    xs = np.ascontiguousarray(x[order].astype(np.float16))  # sorted, f16

    if counts not in _prog_cache:
        _prog_cache[counts] = _build_program(counts)
    nc = _prog_cache[counts]

    z_q = _unpack_zeros(qz_q, Q)
    z_k = _unpack_zeros(qz_k, KV)
    z_v = _unpack_zeros(qz_v, KV)

    SQ, SK = Q // N_CORES, KV // N_CORES
    in_maps = []
    for c in range(N_CORES):
        qs = slice(SQ * c, SQ * (c + 1))
        ks = slice(SK * c, SK * (c + 1))
        wb = np.concatenate(
            [w_base[qs], w_base[Q + SK * c : Q + SK * (c + 1)],
             w_base[Q + KV + SK * c : Q + KV + SK * (c + 1)]], axis=0
        )  # [OS, IN]
        qw = np.concatenate([qw_q[:, qs], qw_k[:, ks], qw_v[:, ks]], axis=1)
        z = np.concatenate([z_q[:, qs], z_k[:, ks], z_v[:, ks]], axis=1)
        sc = np.concatenate(
            [sc_q[:, qs, 0], sc_k[:, ks, 0], sc_v[:, ks, 0]], axis=1
        )  # [D, OS]
        b = np.concatenate(
            [bias[qs], bias[Q + SK * c : Q + SK * (c + 1)],
             bias[Q + KV + SK * c : Q + KV + SK * (c + 1)]]
        )  # [OS]

        wbT = np.ascontiguousarray(wb.T.astype(np.float16))  # [IN, OS]
        qwT = _repack_qw(np.ascontiguousarray(qw))
        # scq[d, h, p, q*OSH + o] = sc[d, h*OSH + o] * 2^18 (per C-quad mult)
        s18 = (sc * float(2.0**18)).astype(np.float16)  # [D, OS]
        scq = np.empty([D, 2, 128, 4 * OSH], np.float16)
        for d in range(D):
            for h in range(2):
                row = np.tile(s18[d, h * OSH : (h + 1) * OSH], 4)
                scq[d, h] = row[None, :]
        scq = np.ascontiguousarray(scq)
        znr2 = (-(z + 1.0) * sc).astype(np.float16)  # [D, OS]
        auxw = np.zeros([3, D * OS], np.float16)
        auxw[0] = np.tile(b.astype(np.float16), D)
        auxw[1] = znr2.reshape(-1)
        auxw[2] = znr2.reshape(-1)

        in_maps.append(
            {
                "x": xs,
                "wbT": wbT,
                "qwT": qwT,
                "scq": scq,
                "auxw": np.ascontiguousarray(auxw),
            }
        )

    import os

    trace = bool(int(os.environ.get("KERNEL_TRACE", "0")))
    res = run_bass_kernel_spmd(
        nc, in_maps, core_ids=list(range(N_CORES)), trace=trace
    )
    kernel._last_results = res

    out = np.zeros([T, OUT], np.float32)
    for c in range(N_CORES):
        rT = np.asarray(res.results[c]["outT"])  # [OS, T_PAD]
        r = rT.T  # [T_PAD, OS]
        cols = np.concatenate(
            [
                np.arange(SQ * c, SQ * (c + 1)),
                np.arange(Q + SK * c, Q + SK * (c + 1)),
                np.arange(Q + KV + SK * c, Q + KV + SK * (c + 1)),
            ]
        )
        out[order[:, None], cols[None, :]] = r
    return out
